# revision 1
# baseline (speedup 1.0000x reference)
"""GAT 2-layer distributed Bass kernel for TRN2 (8 cores).

Table layout (per node-slot, 576B row = 144 f32):
  [asrc 8xf32 | adst 8xf32 | h 256xbf16]
T1 = layer-1 table (written locally by replicated dense phase)
T2 = layer-2 table (AllGather of per-shard cc_in)

Slots: NSLOT = 8 * NTILE * 128. Node->slot permutation balances edge counts
per (core, tile). Slot 0 and slot VB are zero dummies (gather-pad targets).
Views for int16 gather indices: A = rows [0, VA), B = rows [VB, NSLOT),
VA = 32768 (or NSLOT/2 for mini), VB = NSLOT - VA.
"""
import dataclasses
import numpy as np


@dataclasses.dataclass
class Cfg:
    ncores: int = 8
    ntile: int = 49          # dst tiles per core
    nchunk: int = 10         # 128-edge chunks per tile
    na: int = 640            # A-view edge slots per tile (chunks 0..na/128)
    nb: int = 640            # B-view edge slots per tile
    n: int = 50000           # real nodes
    e: int = 400000
    fin: int = 128
    h: int = 8
    c: int = 32              # layer-1 head dim (h*c = 256)
    out: int = 32            # layer-2 head dim
    bt: int = 4              # tiles per gather batch
    use_collective: bool = True
    skip_exp: bool = False
    phases: int = 4          # 1=D1, 2=+E1, 3=+D2, 4=+xfer+E2
    elevel: int = 2          # 0=gathers only, 1=no ACT exp, 2=full
    xbatch: int = 28         # dense node-tiles per x-stream DMA

    @property
    def shslots(self):
        return self.ntile * 128

    @property
    def nslot(self):
        return self.ncores * self.shslots

    @property
    def va(self):
        return min(32768, self.nslot // 2)

    @property
    def vb(self):
        return self.nslot - self.va

    @property
    def d1(self):
        return self.h * self.c      # 256

    @property
    def rec(self):
        return 16 + self.d1 // 2 + 48    # 192 f32 per row (768B, stride %256)


def host_prep(cfg: Cfg, x, edge_index, W1, as1, ad1, b1, W2, as2, ad2, b2):
    N, E = cfg.n, cfg.e
    NC, NT, SH = cfg.ncores, cfg.ntile, cfg.shslots
    src = np.asarray(edge_index[0], dtype=np.int64)
    dst = np.asarray(edge_index[1], dtype=np.int64)
    deg = np.bincount(dst, minlength=N)

    # ---- assign nodes to (core, tile, slot), balancing edge counts ----
    order = np.argsort(-deg, kind="stable")
    core_load = np.zeros(NC, dtype=np.int64)
    core_cnt = np.zeros(NC, dtype=np.int64)
    cap_core = N // NC
    node_core = np.empty(N, dtype=np.int64)
    for nd in order:
        k = np.argmin(np.where(core_cnt < cap_core, core_load, np.iinfo(np.int64).max))
        node_core[nd] = k
        core_load[k] += deg[nd]
        core_cnt[k] += 1

    # reserved dummy slots: slot 0 and slot VB
    rsv = {}
    for s in (0, cfg.vb):
        rsv.setdefault(s // SH, []).append((s % SH) // 128)
    slot2node = np.full(cfg.nslot, -1, dtype=np.int64)
    node_slot = np.empty(N, dtype=np.int64)
    for k in range(NC):
        nodes_k = order[node_core[order] == k]
        tcap = np.full(NT, 128, dtype=np.int64)
        for t in rsv.get(k, []):
            tcap[t] -= 1  # slot-in-tile 0 reserved for the dummy
        tload = np.zeros(NT, dtype=np.int64)
        tcnt = np.zeros(NT, dtype=np.int64)
        tmember = [[] for _ in range(NT)]
        for nd in nodes_k:
            t = np.argmin(np.where(tcnt < tcap, tload, np.iinfo(np.int64).max))
            tmember[t].append(nd)
            tload[t] += deg[nd]
            tcnt[t] += 1
        for t in range(NT):
            s0 = 1 if t in rsv.get(k, []) else 0
            for i, nd in enumerate(tmember[t]):
                s = k * SH + t * 128 + s0 + i
                slot2node[s] = nd
                node_slot[nd] = s

    assert slot2node[0] == -1 and slot2node[cfg.vb] == -1

    # ---- per (core, tile) edge lists with A/B split ----
    sslot = node_slot[src]
    dslot = node_slot[dst]
    ecore = dslot // SH
    etile = (dslot % SH) // 128

    NA, NB, NCH = cfg.na, cfg.nb, cfg.nchunk
    assert NA + NB == NCH * 128 and NA % 128 == 0 and NB % 128 == 0

    idxA = np.zeros((NC, NT, NA), dtype=np.int64)      # src slot, A view
    idxB = np.zeros((NC, NT, NB), dtype=np.int64)      # src slot - VB
    idxD = np.zeros((NC, NT, NA + NB), dtype=np.int64)  # dst slot (raw)
    dloc = np.full((NC, NT, NA + NB), -1.0, dtype=np.float32)
    epos = np.empty(E, dtype=np.int64)  # edge -> (core,tile,pos) flat position

    for k in range(NC):
        for t in range(NT):
            sel = np.nonzero((ecore == k) & (etile == t))[0]
            ss = sslot[sel]
            inA = ss < cfg.va
            inB = ss >= cfg.vb
            flex = inA & inB
            forcedA = inA & ~inB
            forcedB = inB & ~inA
            a_list = list(np.nonzero(forcedA)[0])
            b_list = list(np.nonzero(forcedB)[0])
            for i in np.nonzero(flex)[0]:
                (a_list if len(a_list) < NA else b_list).append(i)
            if len(a_list) > NA or len(b_list) > NB:
                raise RuntimeError(
                    f"tile overflow core{k} tile{t}: {len(a_list)}/{NA} {len(b_list)}/{NB}"
                )
            for p, i in enumerate(a_list):
                e_id = sel[i]
                idxA[k, t, p] = ss[i]
                idxD[k, t, p] = dslot[e_id]
                dloc[k, t, p] = (dslot[e_id] % SH) % 128
                epos[e_id] = ((k * NT + t) * (NA + NB)) + p
            for p, i in enumerate(b_list):
                e_id = sel[i]
                q = NA + p
                idxB[k, t, p] = ss[i] - cfg.vb
                idxD[k, t, q] = dslot[e_id]
                dloc[k, t, q] = (dslot[e_id] % SH) % 128
                epos[e_id] = ((k * NT + t) * (NA + NB)) + q

    # dst-view split by core (shards are entirely in A or in B)
    coreA = (np.arange(NC) * SH + SH - 1) < cfg.va   # entire shard below VA
    idxDA = np.where(coreA[:, None, None], idxD, 0)
    idxDB = np.where(coreA[:, None, None], 0, idxD - cfg.vb)
    # pads in idxD rows of real edges: for coreA pads stay 0 ok; for coreB,
    # pad positions had idxD==0 -> 0 - vb negative! fix: pads -> 0 in both.
    padmask = dloc < 0
    idxDA[padmask] = 0
    idxDB[padmask] = 0

    def pack16(v, width):
        # v [NC, NT, width] int -> [NC, NT, 128, width//16] int16 wrapped+replicated
        assert v.shape[-1] == width and width % 16 == 0
        r = v.reshape(NC, NT, width // 16, 16)
        r = np.transpose(r, (0, 1, 3, 2))  # [NC, NT, 16, width//16]
        r = np.tile(r, (1, 1, 8, 1)).astype(np.int16)
        return np.ascontiguousarray(r)

    pidxA = pack16(idxA, NA)
    pidxB = pack16(idxB, NB)
    pidxDA = pack16(idxDA, NA + NB)
    pidxDB = pack16(idxDB, NA + NB)

    # dstloc [NC, NT, 128, NCH]: position p = j*128 + lane
    dloc_t = np.transpose(dloc.reshape(NC, NT, NCH, 128), (0, 1, 3, 2))
    dloc_t = np.ascontiguousarray(dloc_t.astype(np.float32))

    # validmask [NC, NT, 128, 1]
    vm = (slot2node.reshape(NC, NT, 128) >= 0).astype(np.float32)[..., None]
    vm = np.ascontiguousarray(vm)

    # xT permuted (replicated across cores) [128, NSLOT] bf16
    import ml_dtypes
    xp = np.zeros((cfg.nslot, cfg.fin), dtype=np.float32)
    real = slot2node >= 0
    xp[real] = np.asarray(x, dtype=np.float32)[slot2node[real]]
    xT = np.ascontiguousarray(xp.T).astype(ml_dtypes.bfloat16)

    def fuse(W, asv, adv, hdim):
        Wa = np.einsum("fhc,hc->fh", W.reshape(W.shape[0], cfg.h, hdim), asv)
        Wd = np.einsum("fhc,hc->fh", W.reshape(W.shape[0], cfg.h, hdim), adv)
        return np.concatenate([W, Wa, Wd], axis=1)  # [F, hd*8+16]

    Wall1 = fuse(np.asarray(W1, np.float32), np.asarray(as1, np.float32),
                 np.asarray(ad1, np.float32), cfg.c).astype(ml_dtypes.bfloat16)
    Wall2f = fuse(np.asarray(W2, np.float32), np.asarray(as2, np.float32),
                  np.asarray(ad2, np.float32), cfg.out)
    Wall2 = np.ascontiguousarray(
        Wall2f.reshape(2, 128, Wall2f.shape[1])).astype(ml_dtypes.bfloat16)

    b1t = np.ascontiguousarray(np.tile(np.asarray(b1, np.float32), (128, 1)))
    b2t = np.ascontiguousarray(np.tile(np.asarray(b2, np.float32), (128, 1)))
    iota = np.tile(np.arange(128, dtype=np.float32), (128, 1))
    ident = np.eye(128, dtype=np.float32).astype(ml_dtypes.bfloat16)

    in_maps = []
    for k in range(NC):
        in_maps.append({
            "xT": xT, "Wall1": Wall1, "Wall2": Wall2,
            "b1t": b1t, "b2t": b2t, "iota": iota, "ident": ident,
            "idxA": pidxA[k], "idxB": pidxB[k],
            "idxDA": pidxDA[k], "idxDB": pidxDB[k],
            "dstloc": dloc_t[k], "vmask": vm[k],
        })
    return in_maps, slot2node


class _PhaseDone(Exception):
    def __init__(self, nc):
        self.nc = nc


def build(cfg: Cfg):
    import concourse.bacc as bacc
    import concourse.mybir as mybir
    import concourse.tile as tile
    from concourse import library_config
    from contextlib import ExitStack

    f32 = mybir.dt.float32
    bf16 = mybir.dt.bfloat16
    i16 = mybir.dt.int16
    AOP = mybir.AluOpType
    ACTF = mybir.ActivationFunctionType
    X = mybir.AxisListType.X

    NC, NT, NCH, NA, NB = cfg.ncores, cfg.ntile, cfg.nchunk, cfg.na, cfg.nb
    NAC, NBC = NA // 128, NB // 128
    SH, NSLOT, VA, VB = cfg.shslots, cfg.nslot, cfg.va, cfg.vb
    D1, REC, BT = cfg.d1, cfg.rec, cfg.bt
    HEND = 16 + D1 // 2
    NW = D1 + 16
    EPS = 1e-16

    nc = bacc.Bacc('TRN2', target_bir_lowering=False, debug=False, num_devices=NC)

    xT_d = nc.dram_tensor('xT', [128, NSLOT], bf16, kind='ExternalInput')
    Wall1_d = nc.dram_tensor('Wall1', [128, NW], bf16, kind='ExternalInput')
    Wall2_d = nc.dram_tensor('Wall2', [2, 128, NW], bf16, kind='ExternalInput')
    b1t_d = nc.dram_tensor('b1t', [128, D1], f32, kind='ExternalInput')
    b2t_d = nc.dram_tensor('b2t', [128, cfg.out], f32, kind='ExternalInput')
    iota_d = nc.dram_tensor('iota', [128, 128], f32, kind='ExternalInput')
    ident_d = nc.dram_tensor('ident', [128, 128], bf16, kind='ExternalInput')
    idxA_d = nc.dram_tensor('idxA', [NT, 128, NA // 16], i16, kind='ExternalInput')
    idxB_d = nc.dram_tensor('idxB', [NT, 128, NB // 16], i16, kind='ExternalInput')
    idxDA_d = nc.dram_tensor('idxDA', [NT, 128, (NA + NB) // 16], i16, kind='ExternalInput')
    idxDB_d = nc.dram_tensor('idxDB', [NT, 128, (NA + NB) // 16], i16, kind='ExternalInput')
    dstloc_d = nc.dram_tensor('dstloc', [NT, 128, NCH], f32, kind='ExternalInput')
    vmask_d = nc.dram_tensor('vmask', [NT, 128, 1], f32, kind='ExternalInput')
    out2_d = nc.dram_tensor('out2', [SH, cfg.out], f32, kind='ExternalOutput')
    T1 = nc.dram_tensor('T1', [NSLOT, REC], f32, kind='Internal')
    cc_in = nc.dram_tensor('cc_in', [SH, REC], f32, kind='Internal')
    T2 = nc.dram_tensor('T2', [NSLOT, REC], f32, kind='Internal',
                        addr_space='Shared' if cfg.use_collective else 'Local')

    try:
      with tile.TileContext(nc) as tc, ExitStack() as ctx:
        const = ctx.enter_context(tc.tile_pool(name='const', bufs=1))
        elup = ctx.enter_context(tc.tile_pool(name='elup', bufs=1))
        nc.gpsimd.load_library(library_config.mlp)

        w1 = const.tile([128, NW], bf16)
        nc.sync.dma_start(w1[:], Wall1_d[:])
        w2 = const.tile([128, 2, NW], bf16)
        nc.sync.dma_start(w2[:], Wall2_d[:].rearrange("k p w -> p k w"))
        b1 = const.tile([128, D1], f32)
        nc.sync.dma_start(b1[:], b1t_d[:])
        b2 = const.tile([128, cfg.out], f32)
        nc.sync.dma_start(b2[:], b2t_d[:])
        iot = const.tile([128, 128], f32)
        nc.sync.dma_start(iot[:], iota_d[:])
        idn = const.tile([128, 128], bf16)
        nc.sync.dma_start(idn[:], ident_d[:])

        # ---------- phase D1: replicated dense, writes T1 ----------
        with tc.tile_pool(name='dx', bufs=2) as dx, \
             tc.tile_pool(name='dps', bufs=2, space='PSUM') as dps, \
             tc.tile_pool(name='drow', bufs=3) as drow:
            ng = NSLOT // 128
            for g0 in range(0, ng, cfg.xbatch):
                gb = min(cfg.xbatch, ng - g0)
                xt = dx.tile([128, gb * 128], bf16, tag='xt')
                nc.sync.dma_start(xt[:], xT_d[:, g0 * 128:(g0 + gb) * 128])
                for t in range(gb):
                    ps = dps.tile([128, NW], f32, tag='dps')
                    nc.tensor.matmul(ps[:], xt[:, t * 128:(t + 1) * 128], w1[:],
                                     start=True, stop=True)
                    row = drow.tile([128, REC], f32, tag='row')
                    nc.any.tensor_copy(row[:, 0:16], ps[:, D1:NW])
                    nc.any.tensor_copy(row[:, 16:HEND].bitcast(bf16), ps[:, 0:D1])
                    g = g0 + t
                    nc.sync.dma_start(T1[g * 128:(g + 1) * 128, 0:HEND], row[:, 0:HEND])

        elus = {}

        def edge_phase(layer, T, epilogue):
            pname = f'e{layer}'
            with tc.tile_pool(name=pname + 'g', bufs=2) as gp, \
                 tc.tile_pool(name=pname + 'w', bufs=3) as wp, \
                 tc.tile_pool(name=pname + 'o', bufs=3) as op, \
                 tc.tile_pool(name=pname + 'ps', bufs=4, space='PSUM') as pp:
                hA_src = T[0:VA, 16:HEND].bitcast(bf16)
                hB_src = T[VB:NSLOT, 16:HEND].bitcast(bf16)
                aA_src = T[0:VA, 0:64]
                aB_src = T[VB:NSLOT, 0:64]
                for b0 in range(0, NT, BT):
                    bt = min(BT, NT - b0)
                    ia = gp.tile([128, bt, NA // 16], i16, tag='ia')
                    nc.sync.dma_start(ia[:], idxA_d[b0:b0 + bt].rearrange("t p w -> p t w"))
                    ib = gp.tile([128, bt, NB // 16], i16, tag='ib')
                    nc.sync.dma_start(ib[:], idxB_d[b0:b0 + bt].rearrange("t p w -> p t w"))
                    ida = gp.tile([128, bt, (NA + NB) // 16], i16, tag='ida')
                    nc.sync.dma_start(ida[:], idxDA_d[b0:b0 + bt].rearrange("t p w -> p t w"))
                    idb = gp.tile([128, bt, (NA + NB) // 16], i16, tag='idb')
                    nc.sync.dma_start(idb[:], idxDB_d[b0:b0 + bt].rearrange("t p w -> p t w"))
                    dl = gp.tile([128, bt, NCH], f32, tag='dl')
                    nc.sync.dma_start(dl[:], dstloc_d[b0:b0 + bt].rearrange("t p w -> p t w"))

                    hA = gp.tile([128, bt * NAC, D1], bf16, tag='hA')
                    nc.gpsimd.dma_gather(hA[:], hA_src, ia[:].rearrange("p t w -> p (t w)"),
                                         bt * NA, bt * NA, D1, elem_step=REC * 2, single_packet=False)
                    hB = gp.tile([128, bt * NBC, D1], bf16, tag='hB')
                    nc.gpsimd.dma_gather(hB[:], hB_src, ib[:].rearrange("p t w -> p (t w)"),
                                         bt * NB, bt * NB, D1, elem_step=REC * 2, single_packet=False)
                    aA = gp.tile([128, bt * NAC, 64], f32, tag='aA')
                    nc.gpsimd.dma_gather(aA[:], aA_src, ia[:].rearrange("p t w -> p (t w)"),
                                         bt * NA, bt * NA, 64, elem_step=REC, single_packet=False)
                    aB = gp.tile([128, bt * NBC, 64], f32, tag='aB')
                    nc.gpsimd.dma_gather(aB[:], aB_src, ib[:].rearrange("p t w -> p (t w)"),
                                         bt * NB, bt * NB, 64, elem_step=REC, single_packet=False)
                    dA = gp.tile([128, bt * NCH, 64], f32, tag='dA')
                    nc.gpsimd.dma_gather(dA[:], aA_src, ida[:].rearrange("p t w -> p (t w)"),
                                         bt * (NA + NB), bt * (NA + NB), 64, elem_step=REC, single_packet=False)
                    dB = gp.tile([128, bt * NCH, 64], f32, tag='dB')
                    nc.gpsimd.dma_gather(dB[:], aB_src, idb[:].rearrange("p t w -> p (t w)"),
                                         bt * (NA + NB), bt * (NA + NB), 64, elem_step=REC, single_packet=False)

                    if cfg.elevel == 0:
                        probe = wp.tile([128, 16], f32, tag='probe')
                        nc.vector.tensor_copy(probe[:, 0:4], hA[:, 0, 0:8:2])
                        nc.vector.tensor_tensor(probe[:, 0:4], probe[:, 0:4], aA[:, 0, 0:4], op=AOP.add)
                        nc.vector.tensor_tensor(probe[:, 0:4], probe[:, 0:4], dA[:, 0, 0:4], op=AOP.add)
                        nc.vector.tensor_tensor(probe[:, 0:4], probe[:, 0:4], hB[:, 0, 0:4], op=AOP.add)
                        nc.vector.tensor_tensor(probe[:, 0:4], probe[:, 0:4], aB[:, 0, 0:4], op=AOP.add)
                        nc.vector.tensor_tensor(probe[:, 0:4], probe[:, 0:4], dB[:, 0, 0:4], op=AOP.add)
                        nc.sync.dma_start(dbg1[b0 * 4:b0 * 4 + 128, 0:4], probe[:, 0:4])
                        continue
                    for t in range(bt):
                        tg = b0 + t
                        zb = wp.tile([128, NCH * 8], f32, tag='zb')
                        zA = zb[:, 0:NAC * 8].rearrange("p (b h) -> p b h", b=NAC)
                        nc.vector.tensor_tensor(zA, aA[:, t * NAC:(t + 1) * NAC, 0:8],
                                                dA[:, t * NCH:t * NCH + NAC, 8:16], op=AOP.add)
                        nc.vector.tensor_tensor(zA, zA,
                                                dB[:, t * NCH:t * NCH + NAC, 8:16], op=AOP.add)
                        zB = zb[:, NAC * 8:].rearrange("p (b h) -> p b h", b=NBC)
                        nc.vector.tensor_tensor(zB, aB[:, t * NBC:(t + 1) * NBC, 0:8],
                                                dA[:, t * NCH + NAC:(t + 1) * NCH, 8:16], op=AOP.add)
                        nc.vector.tensor_tensor(zB, zB,
                                                dB[:, t * NCH + NAC:(t + 1) * NCH, 8:16], op=AOP.add)
                        u = wp.tile([128, NCH * 8], f32, tag='u')
                        nc.vector.tensor_scalar(u[:], zb[:], 0.2, None, op0=AOP.mult)
                        nc.vector.tensor_tensor(zb[:], u[:], zb[:], op=AOP.max)
                        p = wp.tile([128, NCH * 8], bf16, tag='p')
                        if cfg.elevel >= 2:
                            nc.scalar.activation(p[:], zb[:], ACTF.Exp)
                        else:
                            nc.vector.tensor_copy(p[:], zb[:])
                        msgA = wp.tile([128, NAC, 8, 32], bf16, tag='msgA')
                        nc.vector.tensor_tensor(
                            msgA[:],
                            hA[:, t * NAC:(t + 1) * NAC, :].rearrange("p b (h c) -> p b h c", h=8),
                            p[:, 0:NAC * 8].rearrange("p (b h) -> p b h ()", b=NAC).to_broadcast([128, NAC, 8, 32]),
                            op=AOP.mult)
                        msgB = wp.tile([128, NBC, 8, 32], bf16, tag='msgB')
                        nc.vector.tensor_tensor(
                            msgB[:],
                            hB[:, t * NBC:(t + 1) * NBC, :].rearrange("p b (h c) -> p b h c", h=8),
                            p[:, NAC * 8:].rearrange("p (b h) -> p b h ()", b=NBC).to_broadcast([128, NBC, 8, 32]),
                            op=AOP.mult)
                        pa = pp.tile([128, D1], f32, tag='pa')
                        pd = pp.tile([128, 8], f32, tag='pd')
                        for j in range(NCH):
                            oh = wp.tile([128, 128], bf16, tag='oh')
                            nc.vector.tensor_scalar(oh[:], iot[:], dl[:, t, j:j + 1], None,
                                                    op0=AOP.is_equal)
                            if j < NAC:
                                rhs = msgA[:, j].rearrange("p h c -> p (h c)")
                            else:
                                rhs = msgB[:, j - NAC].rearrange("p h c -> p (h c)")
                            nc.tensor.matmul(pa[:], oh[:], rhs,
                                             start=(j == 0), stop=(j == NCH - 1))
                            nc.tensor.matmul(pd[:], oh[:], p[:, j * 8:(j + 1) * 8],
                                             start=(j == 0), stop=(j == NCH - 1))
                        epilogue(tg, pa, pd, op)

        def epi1(tg, pa, pd, op):
            d1 = op.tile([128, 8], f32, tag='d1')
            nc.vector.tensor_scalar(d1[:], pd[:], EPS, None, op0=AOP.add)
            r = op.tile([128, 8], f32, tag='r')
            nc.vector.reciprocal(r[:], d1[:])
            o1 = op.tile([128, D1], f32, tag='o1')
            rb = r[:].rearrange("p h -> p h ()").to_broadcast([128, 8, 32])
            nc.vector.tensor_tensor(o1[:].rearrange("p (h c) -> p h c", h=8),
                                    pa[:].rearrange("p (h c) -> p h c", h=8), rb, op=AOP.mult)
            nc.vector.tensor_tensor(o1[:], o1[:], b1[:], op=AOP.add)
            ex = op.tile([128, D1], f32, tag='ex')
            if cfg.elevel >= 2:
                nc.scalar.activation(ex[:], o1[:], ACTF.Exp)
            else:
                nc.vector.tensor_copy(ex[:], o1[:])
            nc.vector.tensor_scalar(ex[:], ex[:], 1.0, 1.0, op0=AOP.min, op1=AOP.subtract)
            rl = op.tile([128, D1], f32, tag='rl')
            nc.vector.tensor_scalar(rl[:], o1[:], 0.0, None, op0=AOP.max)
            et = elup.tile([128, D1], bf16, tag=f'elu{tg}')
            nc.vector.tensor_tensor(et[:], ex[:], rl[:], op=AOP.add)
            elus[tg] = et

        def epi2(tg, pa, pd, op):
            d1 = op.tile([128, 8], f32, tag='d1')
            nc.vector.tensor_scalar(d1[:], pd[:], EPS, None, op0=AOP.add)
            r = op.tile([128, 8], f32, tag='r')
            nc.vector.reciprocal(r[:], d1[:])
            o1 = op.tile([128, D1], f32, tag='o1')
            rb = r[:].rearrange("p h -> p h ()").to_broadcast([128, 8, cfg.out])
            nc.vector.tensor_tensor(o1[:].rearrange("p (h c) -> p h c", h=8),
                                    pa[:].rearrange("p (h c) -> p h c", h=8), rb, op=AOP.mult)
            m = op.tile([128, cfg.out], f32, tag='m')
            nc.vector.reduce_sum(m[:].rearrange("p c -> p c ()"),
                                 o1[:].rearrange("p (h c) -> p c h", h=8), axis=X)
            nc.vector.tensor_scalar(m[:], m[:], 1.0 / cfg.h, None, op0=AOP.mult)
            ob = op.tile([128, cfg.out], f32, tag='ob')
            nc.vector.tensor_tensor(ob[:], m[:], b2[:], op=AOP.add)
            nc.sync.dma_start(out2_d[tg * 128:(tg + 1) * 128, :], ob[:])

        if cfg.phases >= 2:
            edge_phase(1, T1, epi1)

        # ---------- phase D2: shard dense, writes cc_in ----------
        if cfg.phases < 3:
            with tc.tile_pool(name='dout', bufs=2) as dout:
                for t in range(NT):
                    ob = dout.tile([128, cfg.out], f32, tag='ob')
                    nc.vector.memset(ob[:], 0.0)
                    nc.sync.dma_start(out2_d[t * 128:(t + 1) * 128, :], ob[:])
        if cfg.phases >= 3:
          with tc.tile_pool(name='d2', bufs=3) as d2, \
             tc.tile_pool(name='d2ps', bufs=2, space='PSUM') as d2ps:
              vmt = const.tile([128, NT], f32)
              nc.sync.dma_start(vmt[:], vmask_d[:].rearrange("t p o -> p (t o)"))
              for t in range(NT):
                  et = elus[t]
                  ptr = d2ps.tile([128, 2, 128], bf16, tag='ptr')
                  nc.tensor.transpose(ptr[:, 0], et[:, 0:128], idn[:])
                  nc.tensor.transpose(ptr[:, 1], et[:, 128:256], idn[:])
                  lh = d2.tile([128, 2, 128], bf16, tag='lh')
                  nc.any.tensor_copy(lh[:, 0], ptr[:, 0])
                  nc.any.tensor_copy(lh[:, 1], ptr[:, 1])
                  pd2 = d2ps.tile([128, NW], f32, tag='pd2')
                  nc.tensor.matmul(pd2[:], lh[:, 0], w2[:, 0], start=True, stop=False)
                  nc.tensor.matmul(pd2[:], lh[:, 1], w2[:, 1], start=False, stop=True)
                  row = d2.tile([128, REC], f32, tag='crow')
                  nc.vector.memset(row[:, HEND:REC], 0.0)
                  nc.vector.tensor_scalar(row[:, 0:16], pd2[:, D1:NW], vmt[:, t:t + 1], None,
                                          op0=AOP.mult)
                  nc.vector.tensor_scalar(row[:, 16:HEND].bitcast(bf16), pd2[:, 0:D1],
                                          vmt[:, t:t + 1], None, op0=AOP.mult)
                  nc.sync.dma_start(cc_in[t * 128:(t + 1) * 128, :], row[:])

        if cfg.phases == 3:
            with tc.tile_pool(name='dout', bufs=2) as dout:
                for t in range(NT):
                    ob = dout.tile([128, cfg.out], f32, tag='ob')
                    nc.vector.memset(ob[:], 0.0)
                    nc.sync.dma_start(out2_d[t * 128:(t + 1) * 128, :], ob[:])
        if cfg.phases >= 4 and cfg.use_collective:
            nc.gpsimd.collective_compute(
                "AllGather", mybir.AluOpType.bypass,
                ins=[cc_in[:]], outs=[T2[:]],
                replica_groups=[list(range(NC))],
            )
        elif cfg.phases >= 4:
            with tc.tile_pool(name='ccb', bufs=2) as ccb:
                for t in range(NT):
                    bb = ccb.tile([128, REC], f32, tag='bb')
                    nc.sync.dma_start(bb[:], cc_in[t * 128:(t + 1) * 128, :])
                    nc.sync.dma_start(T2[t * 128:(t + 1) * 128, :], bb[:])
                for t in range(NT, NSLOT // 128):
                    bb = ccb.tile([128, REC], f32, tag='bb')
                    nc.vector.memset(bb[:], 0.0)
                    nc.sync.dma_start(T2[t * 128:(t + 1) * 128, :], bb[:])

        if cfg.phases >= 4:
            edge_phase(2, T2, epi2)
    except _PhaseDone:
        pass
    nc.compile()
    return nc


def np_reference(x, edge_index, W1, as1, ad1, b1, W2, as2, ad2, b2):
    """Pure-numpy GAT reference (matches reference.py semantics)."""
    def conv(x, W, asv, adv, bias, src, dst, N, concat):
        H, C = asv.shape
        h = (x @ W).reshape(-1, H, C)
        a_src = np.einsum("nhc,hc->nh", h, asv)
        a_dst = np.einsum("nhc,hc->nh", h, adv)
        e = a_src[src] + a_dst[dst]
        e = np.where(e > 0, e, 0.2 * e)
        emax = np.full((N, H), -np.inf, dtype=np.float64)
        np.maximum.at(emax, dst, e)
        emax = np.where(np.isfinite(emax), emax, 0.0)
        p = np.exp(e - emax[dst])
        denom = np.zeros((N, H), dtype=np.float64)
        np.add.at(denom, dst, p)
        alpha = p / (denom[dst] + 1e-16)
        msg = h[src] * alpha[:, :, None]
        out = np.zeros((N, H, C), dtype=np.float64)
        np.add.at(out, dst, msg)
        if concat:
            out = out.reshape(N, H * C)
        else:
            out = out.mean(axis=1)
        return out + bias

    x = np.asarray(x, np.float64)
    src, dst = edge_index[0], edge_index[1]
    N = x.shape[0]
    h = conv(x, np.asarray(W1, np.float64), np.asarray(as1, np.float64),
             np.asarray(ad1, np.float64), np.asarray(b1, np.float64),
             src, dst, N, True)
    h = np.where(h > 0, h, np.exp(np.minimum(h, 0)) - 1)
    out = conv(h, np.asarray(W2, np.float64), np.asarray(as2, np.float64),
               np.asarray(ad2, np.float64), np.asarray(b2, np.float64),
               src, dst, N, False)
    return out.astype(np.float32)


_CACHE = {}


def kernel(x, edge_index, W1, att_src1, att_dst1, b1, W2, att_src2,
           att_dst2, b2):
    cfg = Cfg()
    in_maps, slot2node = host_prep(cfg, x, edge_index, W1, att_src1,
                                   att_dst1, b1, W2, att_src2, att_dst2, b2)
    if 'nc' not in _CACHE:
        _CACHE['nc'] = build(cfg)
    nc = _CACHE['nc']
    from concourse.bass_utils import run_bass_kernel_spmd
    res = run_bass_kernel_spmd(nc, in_maps, core_ids=list(range(cfg.ncores)))
    full = np.concatenate([res.results[k]['out2'] for k in range(cfg.ncores)],
                          axis=0)
    out = np.zeros((cfg.n, cfg.out), np.float32)
    real = slot2node >= 0
    out[slot2node[real]] = full[real]
    return out



# revision 2
# speedup vs baseline: 15.9050x; 15.9050x over previous
"""GAT 2-layer distributed Bass kernel for TRN2 (8 cores) — v2.

Changes vs baseline:
  - single merged 768B full-row gather per view (was h 512B + a 256B +
    2x dst-a 256B = 1280B/edge in 6 gathers) -> 768B/edge in 2 gathers
  - dst attention term via per-tile adst + one-hot-transpose matmul on PE
    (no per-edge dst gathers)
  - layer-2 dense (D2) fused into the edge-1 epilogue per tile; AllGather
    chunked over 7 tile-groups so it overlaps edge-phase-1 compute
  - T1 writes batched 14 groups/DMA (was 1 DMA per 128-row group)
  - idx/dstloc loads hoisted to one DMA per layer
  - batched out2 writes

Table layout per node-slot row (768B = 384 bf16, RB):
  [asrc 8 | adst 8 | h 256 | pad 112] (all bf16)
T1 = layer-1 table (written locally by replicated dense phase)
T2 = layer-2 table (chunked AllGather of per-shard cc chunks)

Slots: NSLOT = 8 * NTILE * 128. Node->slot permutation balances edge counts
per (core, tile). Slot 0 and slot VB are zero dummies (gather-pad targets).
Views for int16 gather indices: A = rows [0, VA), B = rows [VB, NSLOT),
VA = 32768 (or NSLOT/2 for mini), VB = NSLOT - VA.
"""
import dataclasses
import numpy as np


@dataclasses.dataclass
class Cfg:
    ncores: int = 8
    ntile: int = 49          # dst tiles per core
    nchunk: int = 10         # 128-edge chunks per tile
    na: int = 640            # A-view edge slots per tile (chunks 0..na/128)
    nb: int = 640            # B-view edge slots per tile
    n: int = 50000           # real nodes
    e: int = 400000
    fin: int = 128
    h: int = 8
    c: int = 32              # layer-1 head dim (h*c = 256)
    out: int = 32            # layer-2 head dim
    bt: int = 4              # tiles per gather batch
    use_collective: bool = True
    xbatch: int = 28         # dense node-tiles per x-stream DMA
    wg: int = 14             # T1 row-groups per write DMA
    ccb: int = 7             # tiles per AllGather chunk

    @property
    def shslots(self):
        return self.ntile * 128

    @property
    def nslot(self):
        return self.ncores * self.shslots

    @property
    def va(self):
        return min(32768, self.nslot // 2)

    @property
    def vb(self):
        return self.nslot - self.va

    @property
    def d1(self):
        return self.h * self.c      # 256

    @property
    def rec(self):
        return 16 + self.d1 // 2 + 48    # 192 f32 per row (768B, stride %256)


def host_prep(cfg: Cfg, x, edge_index, W1, as1, ad1, b1, W2, as2, ad2, b2):
    N, E = cfg.n, cfg.e
    NC, NT, SH = cfg.ncores, cfg.ntile, cfg.shslots
    CCB = cfg.ccb
    src = np.asarray(edge_index[0], dtype=np.int64)
    dst = np.asarray(edge_index[1], dtype=np.int64)
    deg = np.bincount(dst, minlength=N)

    def rowid(k, t, p):
        # chunk-major table row order so each chunked AllGather output
        # (ranks x CCB tiles) is one contiguous block of T2
        return ((t // CCB) * (NC * CCB * 128) + k * (CCB * 128)
                + (t % CCB) * 128 + p)

    # ---- assign nodes to (core, tile, slot), balancing edge counts ----
    order = np.argsort(-deg, kind="stable")
    core_load = np.zeros(NC, dtype=np.int64)
    core_cnt = np.zeros(NC, dtype=np.int64)
    cap_core = N // NC
    node_core = np.empty(N, dtype=np.int64)
    for nd in order:
        k = np.argmin(np.where(core_cnt < cap_core, core_load, np.iinfo(np.int64).max))
        node_core[nd] = k
        core_load[k] += deg[nd]
        core_cnt[k] += 1

    # reserved dummy slots: slot 0 and slot VB
    # reserved dummy rows (gather pad targets): table rows 0 and VB.
    # invert rowid to find their (core, tile, lane)
    rsv = {}
    for r in (0, cfg.vb):
        blk = NC * CCB * 128
        c, rr = divmod(r, blk)
        k, rr = divmod(rr, CCB * 128)
        tc, p = divmod(rr, 128)
        rsv.setdefault((k, c * CCB + tc), []).append(p)
    # node -> (core, tile, lane)
    node_k = np.empty(N, dtype=np.int64)
    node_t = np.empty(N, dtype=np.int64)
    node_p = np.empty(N, dtype=np.int64)
    slot2node = np.full(cfg.nslot, -1, dtype=np.int64)
    for k in range(NC):
        nodes_k = order[node_core[order] == k]
        tcap = np.full(NT, 128, dtype=np.int64)
        rsv_t = {t: ps for (kk, t), ps in rsv.items() if kk == k}
        for t, ps in rsv_t.items():
            tcap[t] -= len(ps)
        tload = np.zeros(NT, dtype=np.int64)
        tcnt = np.zeros(NT, dtype=np.int64)
        tmember = [[] for _ in range(NT)]
        for nd in nodes_k:
            t = np.argmin(np.where(tcnt < tcap, tload, np.iinfo(np.int64).max))
            tmember[t].append(nd)
            tload[t] += deg[nd]
            tcnt[t] += 1
        for t in range(NT):
            skip = set(rsv_t.get(t, []))
            lanes = [p for p in range(128) if p not in skip]
            for i, nd in enumerate(tmember[t]):
                p = lanes[i]
                node_k[nd], node_t[nd], node_p[nd] = k, t, p
                slot2node[rowid(k, t, p)] = nd

    assert slot2node[0] == -1 and slot2node[cfg.vb] == -1
    node_row = rowid(node_k, node_t, node_p)

    # ---- per (core, tile) edge lists with A/B split ----
    sslot = node_row[src]
    ecore = node_k[dst]
    etile = node_t[dst]
    dlane = node_p[dst]

    NA, NB, NCH = cfg.na, cfg.nb, cfg.nchunk
    assert NA + NB == NCH * 128 and NA % 128 == 0 and NB % 128 == 0

    idxA = np.zeros((NC, NT, NA), dtype=np.int64)      # src slot, A view
    idxB = np.zeros((NC, NT, NB), dtype=np.int64)      # src slot - VB
    dloc = np.full((NC, NT, NA + NB), -1.0, dtype=np.float32)

    for k in range(NC):
        for t in range(NT):
            sel = np.nonzero((ecore == k) & (etile == t))[0]
            ss = sslot[sel]
            inA = ss < cfg.va
            inB = ss >= cfg.vb
            flex = inA & inB
            forcedA = inA & ~inB
            forcedB = inB & ~inA
            a_list = list(np.nonzero(forcedA)[0])
            b_list = list(np.nonzero(forcedB)[0])
            for i in np.nonzero(flex)[0]:
                (a_list if len(a_list) < NA else b_list).append(i)
            if len(a_list) > NA or len(b_list) > NB:
                raise RuntimeError(
                    f"tile overflow core{k} tile{t}: {len(a_list)}/{NA} {len(b_list)}/{NB}"
                )
            for p, i in enumerate(a_list):
                e_id = sel[i]
                idxA[k, t, p] = ss[i]
                dloc[k, t, p] = dlane[e_id]
            for p, i in enumerate(b_list):
                e_id = sel[i]
                idxB[k, t, p] = ss[i] - cfg.vb
                dloc[k, t, NA + p] = dlane[e_id]

    def pack16(v, width):
        # v [NC, NT, width] int -> [NC, NT, 128, width//16] int16 wrapped+replicated
        assert v.shape[-1] == width and width % 16 == 0
        r = v.reshape(NC, -1, width // 16, 16)
        r = np.transpose(r, (0, 1, 3, 2))  # [NC, NT, 16, width//16]
        r = np.tile(r, (1, 1, 8, 1)).astype(np.int16)
        return np.ascontiguousarray(r)

    pidxA = pack16(idxA, NA)
    pidxB = pack16(idxB, NB)

    # own-shard row indices for the per-tile adst gather (layer 1).
    # out[p, t, :] = row(core k, tile t, lane p): idx[t*128+p]
    idxTA = np.zeros((NC, 1, SH), dtype=np.int64)
    idxTB = np.zeros((NC, 1, SH), dtype=np.int64)
    for k in range(NC):
        for t in range(NT):
            for p in range(128):
                r = rowid(k, t, p)
                if r < cfg.va:
                    idxTA[k, 0, t * 128 + p] = r
                else:
                    idxTB[k, 0, t * 128 + p] = r - cfg.vb
    pidxTA = pack16(idxTA, SH).reshape(NC, 128, SH // 16)
    pidxTB = pack16(idxTB, SH).reshape(NC, 128, SH // 16)

    import ml_dtypes
    # dstloc [NC, NT, 128, NCH]: position p = j*128 + lane; pads -> -1
    dloc_t = np.transpose(dloc.reshape(NC, NT, NCH, 128), (0, 1, 3, 2))
    dloc_t = np.ascontiguousarray(dloc_t.astype(ml_dtypes.bfloat16))

    # validmask [NC, NT, 128, 1] indexed by (core, tile, lane)
    vm = np.zeros((NC, NT, 128, 1), dtype=np.float32)
    for k in range(NC):
        for t in range(NT):
            vm[k, t, :, 0] = slot2node[rowid(k, t, np.arange(128))] >= 0
    vm = np.ascontiguousarray(vm)

    # xT permuted (replicated across cores) [128, NSLOT] bf16;
    # column order == table row order (D1 writes row g*128+p from col g*128+p)
    xp = np.zeros((cfg.nslot, cfg.fin), dtype=np.float32)
    real = slot2node >= 0
    xp[real] = np.asarray(x, dtype=np.float32)[slot2node[real]]
    xT = np.ascontiguousarray(xp.T).astype(ml_dtypes.bfloat16)

    def fuse(W, asv, adv, hdim):
        Wa = np.einsum("fhc,hc->fh", W.reshape(W.shape[0], cfg.h, hdim), asv)
        Wd = np.einsum("fhc,hc->fh", W.reshape(W.shape[0], cfg.h, hdim), adv)
        return np.concatenate([W, Wa, Wd], axis=1)  # [F, hd*8+16]

    Wall1 = fuse(np.asarray(W1, np.float32), np.asarray(as1, np.float32),
                 np.asarray(ad1, np.float32), cfg.c).astype(ml_dtypes.bfloat16)
    Wall2f = fuse(np.asarray(W2, np.float32), np.asarray(as2, np.float32),
                  np.asarray(ad2, np.float32), cfg.out)
    Wall2 = np.ascontiguousarray(
        Wall2f.reshape(2, 128, Wall2f.shape[1])).astype(ml_dtypes.bfloat16)

    b1t = np.ascontiguousarray(np.tile(np.asarray(b1, np.float32), (128, 1)))
    b2t = np.ascontiguousarray(np.tile(np.asarray(b2, np.float32), (128, 1)))
    iota = np.tile(np.arange(128, dtype=np.float32), (128, 1)).astype(ml_dtypes.bfloat16)
    ident = np.eye(128, dtype=np.float32).astype(ml_dtypes.bfloat16)

    # out2 row (k, t*128+p) -> node id (-1 for padding lanes)
    out_node = np.full((NC, SH), -1, dtype=np.int64)
    for k in range(NC):
        for t in range(NT):
            out_node[k, t * 128:(t + 1) * 128] = \
                slot2node[rowid(k, t, np.arange(128))]

    in_maps = []
    for k in range(NC):
        in_maps.append({
            "xT": xT, "Wall1": Wall1, "Wall2": Wall2,
            "b1t": b1t, "b2t": b2t, "iota": iota, "ident": ident,
            "idxA": pidxA[k], "idxB": pidxB[k],
            "idxTA": pidxTA[k], "idxTB": pidxTB[k],
            "dstloc": dloc_t[k], "vmask": vm[k],
        })
    return in_maps, out_node


def build(cfg: Cfg):
    import concourse.bacc as bacc
    import concourse.mybir as mybir
    import concourse.tile as tile
    from concourse import library_config
    from contextlib import ExitStack

    f32 = mybir.dt.float32
    bf16 = mybir.dt.bfloat16
    i16 = mybir.dt.int16
    AOP = mybir.AluOpType
    ACTF = mybir.ActivationFunctionType
    X = mybir.AxisListType.X

    NC, NT, NCH, NA, NB = cfg.ncores, cfg.ntile, cfg.nchunk, cfg.na, cfg.nb
    NAC, NBC = NA // 128, NB // 128
    SH, NSLOT, VA, VB = cfg.shslots, cfg.nslot, cfg.va, cfg.vb
    D1, REC, BT = cfg.d1, cfg.rec, cfg.bt
    HEND = 16 + D1 // 2
    NW = D1 + 16
    # RB defined after dram tensors below
    EPS = 1e-16
    CCB = cfg.ccb
    assert NT % CCB == 0
    NCC = NT // CCB           # number of AllGather chunks

    nc = bacc.Bacc('TRN2', target_bir_lowering=False, debug=False, num_devices=NC)

    xT_d = nc.dram_tensor('xT', [128, NSLOT], bf16, kind='ExternalInput')
    Wall1_d = nc.dram_tensor('Wall1', [128, NW], bf16, kind='ExternalInput')
    Wall2_d = nc.dram_tensor('Wall2', [2, 128, NW], bf16, kind='ExternalInput')
    b1t_d = nc.dram_tensor('b1t', [128, D1], f32, kind='ExternalInput')
    b2t_d = nc.dram_tensor('b2t', [128, cfg.out], f32, kind='ExternalInput')
    iota_d = nc.dram_tensor('iota', [128, 128], bf16, kind='ExternalInput')
    ident_d = nc.dram_tensor('ident', [128, 128], bf16, kind='ExternalInput')
    idxA_d = nc.dram_tensor('idxA', [NT, 128, NA // 16], i16, kind='ExternalInput')
    idxB_d = nc.dram_tensor('idxB', [NT, 128, NB // 16], i16, kind='ExternalInput')
    idxTA_d = nc.dram_tensor('idxTA', [128, SH // 16], i16, kind='ExternalInput')
    idxTB_d = nc.dram_tensor('idxTB', [128, SH // 16], i16, kind='ExternalInput')
    dstloc_d = nc.dram_tensor('dstloc', [NT, 128, NCH], bf16, kind='ExternalInput')
    vmask_d = nc.dram_tensor('vmask', [NT, 128, 1], f32, kind='ExternalInput')
    out2_d = nc.dram_tensor('out2', [SH, cfg.out], f32, kind='ExternalOutput')
    RB = 2 * REC          # row length in bf16 units (384 = 768B)
    T1 = nc.dram_tensor('T1', [NSLOT, RB], bf16, kind='Internal')
    HE = 16 + D1          # written row prefix (bf16 cols)
    ccs = [nc.dram_tensor(f'cc{c}', [CCB * 128, RB], bf16, kind='Internal')
           for c in range(NCC)]
    T2 = nc.dram_tensor('T2', [NSLOT, RB], bf16, kind='Internal',
                        addr_space='Shared' if cfg.use_collective else 'Local')

    with tile.TileContext(nc) as tc, ExitStack() as ctx:
        const = ctx.enter_context(tc.tile_pool(name='const', bufs=1))
        nc.gpsimd.load_library(library_config.mlp)

        w1 = const.tile([128, NW], bf16)
        nc.sync.dma_start(w1[:], Wall1_d[:])
        w2 = const.tile([128, 2, NW], bf16)
        nc.sync.dma_start(w2[:], Wall2_d[:].rearrange("k p w -> p k w"))
        b1 = const.tile([128, D1], f32)
        nc.sync.dma_start(b1[:], b1t_d[:])
        b2 = const.tile([128, cfg.out], f32)
        nc.sync.dma_start(b2[:], b2t_d[:])
        iot = const.tile([128, 128], bf16)
        nc.sync.dma_start(iot[:], iota_d[:])
        idn = const.tile([128, 128], bf16)
        nc.sync.dma_start(idn[:], ident_d[:])
        vmt = const.tile([128, NT], f32)
        nc.sync.dma_start(vmt[:], vmask_d[:].rearrange("t p o -> p (t o)"))


        # ---------- phase D1: replicated dense, writes T1 ----------
        with tc.tile_pool(name='dx', bufs=2) as dx, \
             tc.tile_pool(name='dps', bufs=2, space='PSUM') as dps, \
             tc.tile_pool(name='dstg', bufs=2) as dstg:
            ng = NSLOT // 128
            for g0 in range(0, ng, cfg.xbatch):
                gb = min(cfg.xbatch, ng - g0)
                xt = dx.tile([128, gb * 128], bf16, tag='xt')
                nc.sync.dma_start(xt[:], xT_d[:, g0 * 128:(g0 + gb) * 128])
                for w0 in range(0, gb, cfg.wg):
                    wg = min(cfg.wg, gb - w0)
                    stg = dstg.tile([128, wg, REC], f32, tag=f'stg{wg}')
                    nc.vector.memset(stg[:, :, HEND:REC], 0.0)
                    for t in range(wg):
                        ps = dps.tile([128, NW], f32, tag='dps')
                        nc.tensor.matmul(ps[:], xt[:, (w0 + t) * 128:(w0 + t + 1) * 128],
                                         w1[:], start=True, stop=True)
                        nc.any.tensor_copy(stg[:, t, 0:16], ps[:, D1:NW])
                        nc.any.tensor_copy(stg[:, t, 16:HEND].bitcast(bf16), ps[:, 0:D1])
                    g = g0 + w0
                    nc.sync.dma_start(
                        T1[g * 128:(g + wg) * 128, :].rearrange(
                            "(t p) r -> p t r", p=128),
                        stg[:])

        # ---------- edge phases ----------
        def edge_phase(layer, T, epilogue):
            pname = f'e{layer}'
            with tc.tile_pool(name=pname + 'i', bufs=1) as ip, \
                 tc.tile_pool(name=pname + 'g', bufs=2) as gp, \
                 tc.tile_pool(name=pname + 'w', bufs=3) as wp, \
                 tc.tile_pool(name=pname + 'o', bufs=3) as op, \
                 tc.tile_pool(name=pname + 'oh', bufs=2) as ohp, \
                 tc.tile_pool(name=pname + 'ps', bufs=2, space='PSUM') as pp:
                rowA_src = T[0:VA, :]
                rowB_src = T[VB:NSLOT, :]

                # layer-wide idx/dstloc loads
                iaL = ip.tile([128, NT, NA // 16], i16)
                nc.sync.dma_start(iaL[:], idxA_d[:].rearrange("t p w -> p t w"))
                ibL = ip.tile([128, NT, NB // 16], i16)
                nc.sync.dma_start(ibL[:], idxB_d[:].rearrange("t p w -> p t w"))
                dlL = ip.tile([128, NT, NCH], bf16)
                nc.sync.dma_start(dlL[:], dstloc_d[:].rearrange("t p w -> p t w"))

                # per-tile adst [128 slot, NT, 8] bf16
                ad = ip.tile([128, NT, 8], bf16)
                if layer == 1:
                    with tc.tile_pool(name='adg', bufs=1) as adg:
                        ita = adg.tile([128, SH // 16], i16)
                        nc.sync.dma_start(ita[:], idxTA_d[:])
                        itb = adg.tile([128, SH // 16], i16)
                        nc.sync.dma_start(itb[:], idxTB_d[:])
                        aga = adg.tile([128, NT, 128], bf16)
                        nc.gpsimd.dma_gather(aga[:], T[0:VA, 0:128], ita[:],
                                             SH, SH, 128, elem_step=RB,
                                             single_packet=False)
                        agb = adg.tile([128, NT, 128], bf16)
                        nc.gpsimd.dma_gather(agb[:], T[VB:NSLOT, 0:128], itb[:],
                                             SH, SH, 128, elem_step=RB,
                                             single_packet=False)
                        nc.vector.tensor_tensor(ad[:], aga[:, :, 8:16],
                                                agb[:, :, 8:16], op=AOP.add)
                else:
                    for cci in range(NCC):
                        nc.sync.dma_start(
                            ad[:, cci * CCB:(cci + 1) * CCB, :],
                            ccs[cci][:, 8:16].rearrange("(t p) a -> p t a", p=128))

                for b0 in range(0, NT, BT):
                    bt = min(BT, NT - b0)
                    gA = gp.tile([128, bt * NAC, RB], bf16, tag='gA')
                    nc.gpsimd.dma_gather(
                        gA[:], rowA_src,
                        iaL[:, b0:b0 + bt, :].rearrange("p t w -> p (t w)"),
                        bt * NA, bt * NA, RB, single_packet=False)
                    gB = gp.tile([128, bt * NBC, RB], bf16, tag='gB')
                    nc.gpsimd.dma_gather(
                        gB[:], rowB_src,
                        ibL[:, b0:b0 + bt, :].rearrange("p t w -> p (t w)"),
                        bt * NB, bt * NB, RB, single_packet=False)

                    for t in range(bt):
                        tg = b0 + t
                        # one-hots for all chunks of this tile: oh[e, j, slot]
                        ohs = wp.tile([128, NCH, 128], bf16, tag='ohs')
                        nc.vector.tensor_tensor(
                            ohs[:],
                            iot[:].rearrange("p f -> p () f").to_broadcast(
                                [128, NCH, 128]),
                            dlL[:, tg, :].rearrange("p j -> p j ()").to_broadcast(
                                [128, NCH, 128]),
                            op=AOP.is_equal)
                        # zdst[e, j*8+h] via transpose(oh) @ adst_tile
                        # pdz cols 0:NCH*8 hold zdst, cols NCH*8:+8 hold pd
                        pdz = pp.tile([128, NCH * 8 + 8], f32, tag='pdz')
                        ohTs = ohp.tile([128, NCH, 128], bf16, tag='ohTs')
                        half = NCH // 2
                        for hb in range(2):
                            j0 = hb * half
                            jn = half if hb == 0 else NCH - half
                            tps = pp.tile([128, half, 128], bf16, tag='tps')
                            for j in range(jn):
                                nc.tensor.transpose(tps[:, j, :],
                                                    ohs[:, j0 + j, :], idn[:])
                            nc.scalar.copy(ohTs[:, j0:j0 + jn, :], tps[:, 0:jn, :])
                        for j in range(NCH):
                            nc.tensor.matmul(pdz[:, j * 8:(j + 1) * 8],
                                             ohTs[:, j, :],
                                             ad[:, tg, :], start=True, stop=True)
                        # z = asrc[src] + adst[dst]; leaky; exp
                        zb = wp.tile([128, NCH * 8], f32, tag='zb')
                        nc.vector.tensor_tensor(
                            zb[:, 0:NAC * 8].rearrange("p (b h) -> p b h", b=NAC),
                            gA[:, t * NAC:(t + 1) * NAC, 0:8],
                            pdz[:, 0:NAC * 8].rearrange("p (b h) -> p b h", b=NAC),
                            op=AOP.add)
                        nc.vector.tensor_tensor(
                            zb[:, NAC * 8:NCH * 8].rearrange("p (b h) -> p b h", b=NBC),
                            gB[:, t * NBC:(t + 1) * NBC, 0:8],
                            pdz[:, NAC * 8:NCH * 8].rearrange("p (b h) -> p b h", b=NBC),
                            op=AOP.add)
                        u = wp.tile([128, NCH * 8], f32, tag='u')
                        nc.vector.tensor_scalar(u[:], zb[:], 0.2, None, op0=AOP.mult)
                        nc.vector.tensor_tensor(zb[:], u[:], zb[:], op=AOP.max)
                        p = wp.tile([128, NCH * 8], bf16, tag='p')
                        nc.scalar.activation(p[:], zb[:], ACTF.Exp)
                        # msg = h[src] * p
                        msgA = wp.tile([128, NAC, 8, 32], bf16, tag='msgA')
                        nc.vector.tensor_tensor(
                            msgA[:],
                            gA[:, t * NAC:(t + 1) * NAC, 16:16 + D1].rearrange(
                                "p b (h c) -> p b h c", h=8),
                            p[:, 0:NAC * 8].rearrange(
                                "p (b h) -> p b h ()", b=NAC).to_broadcast(
                                [128, NAC, 8, 32]),
                            op=AOP.mult)
                        msgB = wp.tile([128, NBC, 8, 32], bf16, tag='msgB')
                        nc.vector.tensor_tensor(
                            msgB[:],
                            gB[:, t * NBC:(t + 1) * NBC, 16:16 + D1].rearrange(
                                "p b (h c) -> p b h c", h=8),
                            p[:, NAC * 8:].rearrange(
                                "p (b h) -> p b h ()", b=NBC).to_broadcast(
                                [128, NBC, 8, 32]),
                            op=AOP.mult)
                        # scatter to dst slots (pd accumulates in pdz tail cols)
                        pa = pp.tile([128, D1], f32, tag='pa')
                        for j in range(NCH):
                            if j < NAC:
                                rhs = msgA[:, j].rearrange("p h c -> p (h c)")
                            else:
                                rhs = msgB[:, j - NAC].rearrange("p h c -> p (h c)")
                            nc.tensor.matmul(pa[:], ohs[:, j, :], rhs,
                                             start=(j == 0), stop=(j == NCH - 1))
                            nc.tensor.matmul(pdz[:, NCH * 8:NCH * 8 + 8],
                                             ohs[:, j, :], p[:, j * 8:(j + 1) * 8],
                                             start=(j == 0), stop=(j == NCH - 1))
                        epilogue(tg, pa, pdz[:, NCH * 8:NCH * 8 + 8], op)

        # ---------- epilogues ----------
        ccstage = {}

        def epi1(tg, pa, pd, op):
            d1 = op.tile([128, 8], f32, tag='d1')
            nc.vector.tensor_scalar(d1[:], pd, EPS, None, op0=AOP.add)
            r = op.tile([128, 8], f32, tag='r')
            nc.vector.reciprocal(r[:], d1[:])
            o1 = op.tile([128, D1], f32, tag='o1')
            rb = r[:].rearrange("p h -> p h ()").to_broadcast([128, 8, 32])
            nc.vector.tensor_tensor(o1[:].rearrange("p (h c) -> p h c", h=8),
                                    pa[:].rearrange("p (h c) -> p h c", h=8), rb,
                                    op=AOP.mult)
            nc.vector.tensor_tensor(o1[:], o1[:], b1[:], op=AOP.add)
            ex = op.tile([128, D1], f32, tag='ex')
            nc.scalar.activation(ex[:], o1[:], ACTF.Exp)
            nc.vector.tensor_scalar(ex[:], ex[:], 1.0, 1.0, op0=AOP.min,
                                    op1=AOP.subtract)
            rl = op.tile([128, D1], f32, tag='rl')
            nc.vector.tensor_scalar(rl[:], o1[:], 0.0, None, op0=AOP.max)
            et = op.tile([128, D1], bf16, tag='et')
            nc.vector.tensor_tensor(et[:], ex[:], rl[:], op=AOP.add)
            # ---- fused D2: h2 row for this tile -> cc chunk staging ----
            lh = op.tile([128, 2, 128], bf16, tag='lh')
            for half in range(2):
                ptr = pp.tile([128, 128], bf16, tag='tps')
                nc.tensor.transpose(ptr[:], et[:, half * 128:(half + 1) * 128],
                                    idn[:])
                nc.scalar.copy(lh[:, half], ptr[:])
            pd2 = pp.tile([128, NW], f32, tag='pd2')
            nc.tensor.matmul(pd2[:], lh[:, 0], w2[:, 0], start=True, stop=False)
            nc.tensor.matmul(pd2[:], lh[:, 1], w2[:, 1], start=False, stop=True)
            cci, cto = tg // CCB, tg % CCB
            if cto == 0:
                ccstage[cci] = op.tile([128, CCB, RB], bf16, tag='ccstg', name='ccstg')
                nc.vector.memset(ccstage[cci][:, :, HE:RB], 0.0)
            row = ccstage[cci]
            nc.vector.tensor_scalar(row[:, cto, 0:16], pd2[:, D1:NW],
                                    vmt[:, tg:tg + 1], None, op0=AOP.mult)
            nc.vector.tensor_scalar(row[:, cto, 16:16 + D1], pd2[:, 0:D1],
                                    vmt[:, tg:tg + 1], None, op0=AOP.mult)
            if cto == CCB - 1:
                nc.sync.dma_start(
                    ccs[cci][:].rearrange("(t p) r -> p t r", p=128), row[:])
                del ccstage[cci]
                if cfg.use_collective:
                    blk = NC * CCB * 128
                    nc.gpsimd.collective_compute(
                        "AllGather", mybir.AluOpType.bypass,
                        ins=[ccs[cci][:]],
                        outs=[T2[cci * blk:(cci + 1) * blk, :]],
                        replica_groups=[list(range(NC))],
                    )

        outstage = {}

        def epi2(tg, pa, pd, op):
            d1 = op.tile([128, 8], f32, tag='d1')
            nc.vector.tensor_scalar(d1[:], pd, EPS, None, op0=AOP.add)
            r = op.tile([128, 8], f32, tag='r')
            nc.vector.reciprocal(r[:], d1[:])
            o1 = op.tile([128, D1], f32, tag='o1')
            rb = r[:].rearrange("p h -> p h ()").to_broadcast([128, 8, cfg.out])
            nc.vector.tensor_tensor(o1[:].rearrange("p (h c) -> p h c", h=8),
                                    pa[:].rearrange("p (h c) -> p h c", h=8), rb,
                                    op=AOP.mult)
            m = op.tile([128, cfg.out], f32, tag='m')
            nc.vector.reduce_sum(m[:].rearrange("p c -> p c ()"),
                                 o1[:].rearrange("p (h c) -> p c h", h=8), axis=X)
            cci, cto = tg // CCB, tg % CCB
            if cto == 0:
                outstage[cci] = op.tile([128, CCB, cfg.out], f32, tag='ostg', name='ostg')
            ob = outstage[cci]
            nc.vector.tensor_scalar(ob[:, cto, :], m[:], 1.0 / cfg.h, None,
                                    op0=AOP.mult)
            nc.vector.tensor_tensor(ob[:, cto, :], ob[:, cto, :], b2[:],
                                    op=AOP.add)
            if cto == CCB - 1:
                nc.sync.dma_start(
                    out2_d[cci * CCB * 128:(cci + 1) * CCB * 128, :].rearrange(
                        "(t p) c -> p t c", p=128), ob[:])
                del outstage[cci]

        edge_phase(1, T1, epi1)

        if not cfg.use_collective:
            # timing-sim-only stand-in for the AllGather: copy local chunks to
            # the core-0 block of T2 (values wrong cross-core, timing close)
            with tc.tile_pool(name='ccb', bufs=2) as ccbp:
                for cci in range(NCC):
                    bb = ccbp.tile([128, CCB, RB], bf16, tag='bb')
                    nc.sync.dma_start(
                        bb[:], ccs[cci][:].rearrange("(t p) r -> p t r", p=128))
                    nc.sync.dma_start(
                        T2[cci * CCB * 128:(cci + 1) * CCB * 128, :].rearrange(
                            "(t p) r -> p t r", p=128), bb[:])

        edge_phase(2, T2, epi2)

    nc.compile()
    return nc


def np_reference(x, edge_index, W1, as1, ad1, b1, W2, as2, ad2, b2):
    """Pure-numpy GAT reference (matches reference.py semantics)."""
    def conv(x, W, asv, adv, bias, src, dst, N, concat):
        H, C = asv.shape
        h = (x @ W).reshape(-1, H, C)
        a_src = np.einsum("nhc,hc->nh", h, asv)
        a_dst = np.einsum("nhc,hc->nh", h, adv)
        e = a_src[src] + a_dst[dst]
        e = np.where(e > 0, e, 0.2 * e)
        emax = np.full((N, H), -np.inf, dtype=np.float64)
        np.maximum.at(emax, dst, e)
        emax = np.where(np.isfinite(emax), emax, 0.0)
        p = np.exp(e - emax[dst])
        denom = np.zeros((N, H), dtype=np.float64)
        np.add.at(denom, dst, p)
        alpha = p / (denom[dst] + 1e-16)
        msg = h[src] * alpha[:, :, None]
        out = np.zeros((N, H, C), dtype=np.float64)
        np.add.at(out, dst, msg)
        if concat:
            out = out.reshape(N, H * C)
        else:
            out = out.mean(axis=1)
        return out + bias

    x = np.asarray(x, np.float64)
    src, dst = edge_index[0], edge_index[1]
    N = x.shape[0]
    h = conv(x, np.asarray(W1, np.float64), np.asarray(as1, np.float64),
             np.asarray(ad1, np.float64), np.asarray(b1, np.float64),
             src, dst, N, True)
    h = np.where(h > 0, h, np.exp(np.minimum(h, 0)) - 1)
    out = conv(h, np.asarray(W2, np.float64), np.asarray(as2, np.float64),
               np.asarray(ad2, np.float64), np.asarray(b2, np.float64),
               src, dst, N, False)
    return out.astype(np.float32)


_CACHE = {}


def kernel(x, edge_index, W1, att_src1, att_dst1, b1, W2, att_src2,
           att_dst2, b2):
    cfg = Cfg()
    in_maps, out_node = host_prep(cfg, x, edge_index, W1, att_src1,
                                  att_dst1, b1, W2, att_src2, att_dst2, b2)
    if 'nc' not in _CACHE:
        _CACHE['nc'] = build(cfg)
    nc = _CACHE['nc']
    from concourse.bass_utils import run_bass_kernel_spmd
    res = run_bass_kernel_spmd(nc, in_maps, core_ids=list(range(cfg.ncores)))
    full = np.concatenate([res.results[k]['out2'] for k in range(cfg.ncores)],
                          axis=0)
    flat = out_node.reshape(-1)
    out = np.zeros((cfg.n, cfg.out), np.float32)
    real = flat >= 0
    out[flat[real]] = full[real]
    return out
